# revision 13
# baseline (speedup 1.0000x reference)
"""ConvLSTM (B=4, T=8, C=HID=256, H=W=32, 3x3 SAME convs) on 8 TRN2 NeuronCores.

Sharding: data-parallel over batch (4) x spatial halves of H (2) = 8 cores,
zero inter-core communication. Each core computes its half's rows plus a
shrinking halo margin: at step t it computes 23-t rows; wrong values erode
inward from the un-owned edge at 1 row/step, leaving exactly the owned 16
rows correct after T=8 steps. Upper halves are row-flipped host-side (with
dy-flipped kernels) so all 8 cores run the same SPMD instruction stream.

Compute: 1D Winograd F(2,3) along W for both convs at every step: per
output-channel octile, 3(dy) x 4(pos) matmuls over K=ic accumulate four
position planes M_p, and VectorE applies A^T ([m0+m1+m2, m1-m2-m3]) to
produce the 16 even/odd column pairs - 1.5x fewer PE columns than direct
conv, and every step fits a single 512-col PSUM chunk.

Precision: steps 0..3 quantize V (data transform) and U (weight transform)
to fp8(e4m3) and run DoubleRow matmuls (ic-pair, K=256, 2x PE rate); steps
4..7 run bf16. fp8 errors injected at early steps decay through the forget
gate; simulated end-to-end rel err ~1.5e-2 vs the 2e-2 budget. Scales keep
e4m3 in range (x*16/wx*512, h*8/wh*1024 - both products 8192, undone by the
activation scale). The h data transform runs on VectorE (fp8 steps) or
GpSimd (bf16 steps); the x transform is precomputed host-side.
"""
import numpy as np
import ml_dtypes
from contextlib import ExitStack

import concourse.bass as bass
import concourse.tile as tile
from concourse import bacc, mybir
from concourse.bass_utils import run_bass_kernel_spmd

F8 = mybir.dt.float8e4
BF16 = mybir.dt.bfloat16
F32 = mybir.dt.float32
AF = mybir.ActivationFunctionType
ALU = mybir.AluOpType
DR = mybir.MatmulPerfMode.DoubleRow

N_CORES = 8
T = 8
NF8 = 4            # steps 0..NF8-1 run fp8 Winograd; the rest bf16 Winograd
ROWS = 26          # h plane rows: p=0 is the y=-1 zero row, p=1..24 = y=0..23
WC = 34            # padded width
PLANE = ROWS * WC  # 884
CROWS = 23
CPL = CROWS * 32
VR8 = 25           # V rows, fp8 steps (t=0 reads dy..dy+22, dy<=2)
VR16 = 21          # V rows, bf16 steps (t=4 reads dy..dy+18)

XS, WXS = 16.0, 512.0
HS, WHS = 8.0, 1024.0
DESCALE = 1.0 / 8192.0

_cache = {}


def _build_nc():
    nc = bacc.Bacc("TRN2", target_bir_lowering=False, debug=False,
                   num_devices=N_CORES)
    vx8_d = nc.dram_tensor("vx8", [NF8, 128, 2 * 4 * VR8 * 16], F8,
                           kind="ExternalInput").ap()
    vx16_d = nc.dram_tensor("vx16", [T - NF8, 128, 2 * 4 * VR16 * 16], BF16,
                            kind="ExternalInput").ap()
    u8_d = nc.dram_tensor("u8", [24, 128, 2, 1024], F8, kind="ExternalInput").ap()
    u16_d = nc.dram_tensor("u16", [48, 128, 1024], BF16, kind="ExternalInput").ap()
    b_d = nc.dram_tensor("bias", [128, 8], F32, kind="ExternalInput").ap()
    out_d = nc.dram_tensor("hout", [2, 128, 512], F32, kind="ExternalOutput").ap()

    with tile.TileContext(nc) as tc, ExitStack() as ctx:
        wp = ctx.enter_context(tc.tile_pool(name="wp", bufs=1))
        vxp8 = ctx.enter_context(tc.tile_pool(name="vxp8", bufs=2))
        vxp16 = ctx.enter_context(tc.tile_pool(name="vxp16", bufs=2))
        vhp8 = ctx.enter_context(tc.tile_pool(name="vhp8", bufs=1))
        vhp16 = ctx.enter_context(tc.tile_pool(name="vhp16", bufs=1))
        hp = ctx.enter_context(tc.tile_pool(name="hp", bufs=1))
        cp = ctx.enter_context(tc.tile_pool(name="cp", bufs=1))
        bp = ctx.enter_context(tc.tile_pool(name="bp", bufs=1))
        gp = ctx.enter_context(tc.tile_pool(name="gp", bufs=8))
        zp = ctx.enter_context(tc.tile_pool(name="zp", bufs=2))
        wtp = ctx.enter_context(tc.tile_pool(name="wtp", bufs=5))
        tp = ctx.enter_context(tc.tile_pool(name="tp", bufs=2))
        pp = ctx.enter_context(tc.tile_pool(name="pp", bufs=8, space="PSUM"))

        bt = bp.tile([128, 8], F32, tag="bias")
        nc.sync.dma_start(bt[:], b_d[:])

        h16a = hp.tile([128, 2 * PLANE], BF16, tag="h16a")
        h16b = hp.tile([128, 2 * PLANE], BF16, tag="h16b")
        hf = hp.tile([128, 1024], F32, tag="hf")
        ct = cp.tile([128, 2 * CPL], F32, tag="c")
        nc.vector.memset(ct[:], 0.0)
        nc.vector.memset(h16a[:], 0.0)
        nc.vector.memset(h16b[:], 0.0)

        vx0 = vxp8.tile([128, 2 * 4 * VR8 * 16], F8, tag="vx8")
        nc.gpsimd.dma_start(vx0[:], vx8_d[0])

        # fp8 weight tiles, one per (dy, pos). x-conv i/o/g columns first
        # (t=0 skips f octiles and h-convs), issued in consumption order.
        u8x = [wp.tile([128, 2, 768], F8, tag=f"u8x{j}", name=f"u8x{j}")
               for j in range(12)]
        u8xf = [wp.tile([128, 2, 256], F8, tag=f"u8xf{j}", name=f"u8xf{j}")
                for j in range(12)]
        u8h = [wp.tile([128, 2, 1024], F8, tag=f"u8h{j}", name=f"u8h{j}")
               for j in range(12)]
        for j in range(12):
            nc.sync.dma_start(u8x[j][:], u8_d[j][:, :, :768])
        for j in range(12):
            nc.sync.dma_start(u8h[j][:], u8_d[12 + j])
        for j in range(12):
            nc.sync.dma_start(u8xf[j][:], u8_d[j][:, :, 768:])
        u16 = [wp.tile([128, 1024], BF16, tag=f"u16_{j}", name=f"u16_{j}")
               for j in range(48)]
        for j in range(48):
            nc.sync.dma_start(u16[j][:], u16_d[j])

        def u8slice(cv, dy, pos, o):
            j = dy * 4 + pos
            if cv == 0:
                if o < 6:
                    return u8x[j][:, :, o * 128:(o + 1) * 128]
                return u8xf[j][:, :, (o - 6) * 128:(o - 5) * 128]
            return u8h[j][:, :, o * 128:(o + 1) * 128]

        def u16slice(cv, dy, pos, it, o):
            j = ((cv * 3 + dy) * 4 + pos) * 2 + it
            return u16[j][:, o * 128:(o + 1) * 128]

        hbufs = [h16a, h16b]

        for t in range(T):
            fp8 = t < NF8
            r = 23 - t
            n, n2 = r * 32, r * 16
            VR = VR8 if fp8 else VR16
            if t == 0:
                vx = vx0
            elif fp8:
                vx = vxp8.tile([128, 2 * 4 * VR8 * 16], F8, tag="vx8")
                nc.gpsimd.dma_start(vx[:], vx8_d[t])
            else:
                vx = vxp16.tile([128, 2 * 4 * VR16 * 16], BF16, tag="vx16")
                nc.gpsimd.dma_start(vx[:], vx16_d[t - NF8])
            vxv = vx[:].rearrange("p (i s v j) -> p i s v j", i=2, s=4, v=VR, j=16)

            h_in = hbufs[t % 2]
            h_out = hbufs[(t + 1) % 2] if t < T - 1 else None

            # data transform for the h-conv: V = B^T h per 4-col window
            if t > 0:
                if fp8:
                    vh = vhp8.tile([128, 2 * 4 * VR8 * 16], F8, tag="vh8")
                else:
                    vh = vhp16.tile([128, 2 * 4 * VR16 * 16], BF16, tag="vh16")
                vhv = vh[:].rearrange("p (i s v j) -> p i s v j", i=2, s=4, v=VR, j=16)
                hw = h_in[:].rearrange("p (i v c two) -> p i v c two",
                                       i=2, v=ROWS, c=17, two=2)
                d0 = hw[:, :, 0:VR, 0:16, 0]
                d1 = hw[:, :, 0:VR, 0:16, 1]
                d2 = hw[:, :, 0:VR, 1:17, 0]
                d3 = hw[:, :, 0:VR, 1:17, 1]
                eng = nc.vector if fp8 else nc.gpsimd
                eng.tensor_sub(vhv[:, :, 0], d0, d2)
                eng.tensor_add(vhv[:, :, 1], d1, d2)
                eng.tensor_sub(vhv[:, :, 2], d2, d1)
                eng.tensor_sub(vhv[:, :, 3], d1, d3)

            # final h tile stays parity-deinterleaved; the host re-interleaves
            hov = (None if h_out is not None else
                   hf[:].rearrange("p (i e v j) -> p i e v j",
                                   i=2, e=2, v=16, j=16))

            octs = [0, 1, 2, 3, 4, 5] if t == 0 else list(range(8))

            def x_mms(o, ps4):
                for dy in range(3):
                    for pos in range(4):
                        if fp8:
                            nc.tensor.matmul(
                                ps4[pos][:], u8slice(0, dy, pos, o),
                                vxv[:, :, pos, dy:dy + r, :],
                                start=(dy == 0), stop=(t == 0 and dy == 2),
                                perf_mode=DR, skip_group_check=True)
                        else:
                            for it in range(2):
                                nc.tensor.matmul(
                                    ps4[pos][:], u16slice(0, dy, pos, it, o),
                                    vxv[:, it, pos, dy:dy + r, :],
                                    start=(dy == 0 and it == 0), stop=False,
                                    skip_group_check=True)

            def h_mms(o, ps4):
                for dy in range(3):
                    for pos in range(4):
                        if fp8:
                            nc.tensor.matmul(
                                ps4[pos][:], u8slice(1, dy, pos, o),
                                vhv[:, :, pos, dy:dy + r, :],
                                start=False, stop=(dy == 2),
                                perf_mode=DR, skip_group_check=True)
                        else:
                            for it in range(2):
                                nc.tensor.matmul(
                                    ps4[pos][:], u16slice(1, dy, pos, it, o),
                                    vhv[:, it, pos, dy:dy + r, :],
                                    start=False, stop=(dy == 2 and it == 1),
                                    skip_group_check=True)

            gts = {}

            def drain(o, ps4):
                # z = A^T M: z_even = m0+m1+m2, z_odd = m1-m2-m3. Gates, z
                # and c all live in parity-deinterleaved layout (even block
                # then odd block) so every op here is flat/contiguous; only
                # the h-plane write re-interleaves. One PSUM operand per DVE
                # op: m1/m2 staged through ScalarE; t23 runs on GpSimd.
                zt = zp.tile([128, n], BF16, tag="z")
                s1 = wtp.tile([128, n2], BF16, tag="t01")
                s2 = wtp.tile([128, n2], BF16, tag="t01")
                t01 = wtp.tile([128, n2], BF16, tag="t01")
                t23 = wtp.tile([128, n2], BF16, tag="t01")
                nc.scalar.activation(s1[:], ps4[1][:], AF.Copy)
                nc.scalar.activation(s2[:], ps4[2][:], AF.Copy)
                nc.vector.tensor_add(t01[:], s1[:], ps4[0][:])
                nc.vector.tensor_add(zt[:, :n2], t01[:], ps4[2][:])
                nc.gpsimd.tensor_sub(t23[:], s1[:], s2[:])
                nc.vector.tensor_sub(zt[:, n2:], t23[:], ps4[3][:])
                gt = gp.tile([128, n], BF16, tag="g")
                gts[o] = gt
                func = AF.Relu if o in (4, 5) else AF.Sigmoid
                nc.scalar.activation(gt[:], zt[:], func, bias=bt[:, o:o + 1],
                                     scale=DESCALE if fp8 else 1.0)

            def alloc4():
                return [pp.tile([128, n2], F32, tag="ps", name=f"ps{i}")
                        for i in range(4)]

            ps_map = {}
            ps_map[octs[0]] = alloc4()
            ps_map[octs[1]] = alloc4()
            x_mms(octs[0], ps_map[octs[0]])
            x_mms(octs[1], ps_map[octs[1]])
            for idx, o in enumerate(octs):
                if t > 0:
                    h_mms(o, ps_map[o])
                drain(o, ps_map[o])
                del ps_map[o]
                if idx + 2 < len(octs):
                    nxt = octs[idx + 2]
                    ps_map[nxt] = alloc4()
                    x_mms(nxt, ps_map[nxt])

            # state update; halves run on different engines in parallel.
            # c holds the parity-deinterleaved layout at fixed stride;
            # gate/cr tiles are step-sized so views reconcile the strides.
            ctv = ct[:].rearrange("p (h e v j) -> p h e v j",
                                  h=2, e=2, v=CROWS, j=16)
            for hi in range(2):
                eng = nc.vector if hi == 0 else nc.gpsimd
                gi, go, gg = gts[0 + hi], gts[2 + hi], gts[4 + hi]

                def dv(x):
                    return x[:].rearrange("p (e v j) -> p e v j",
                                          e=2, v=r, j=16)
                cs = ctv[:, hi, :, 0:r, :]
                if t == 0:
                    eng.tensor_mul(cs, dv(gi), dv(gg))
                else:
                    gf = gts[6 + hi]
                    eng.tensor_mul(gg[:], gi[:], gg[:])
                    eng.tensor_mul(cs, dv(gf), cs)
                    eng.tensor_add(cs, cs, dv(gg))
                cr = tp.tile([128, n], BF16, tag="cr")
                if t < NF8 - 1:
                    # next step's conv consumes h in fp8 scaled by HS
                    eng.tensor_scalar(dv(cr), cs, 0.0, HS, ALU.max, ALU.mult)
                else:
                    eng.tensor_scalar_max(dv(cr), cs, 0.0)
                crv, gov = dv(cr), dv(go)
                if t == T - 1:
                    for e in range(2):
                        eng.tensor_mul(hov[:, hi, e, :, :],
                                       gov[:, e], crv[:, e])
                else:
                    hw2 = h_out[:].rearrange("p (i v ch two) -> p i v ch two",
                                             i=2, v=ROWS, ch=17, two=2)
                    eng.tensor_mul(hw2[:, hi, 1:1 + r, 0:16, 1],
                                   gov[:, 0], crv[:, 0])
                    eng.tensor_mul(hw2[:, hi, 1:1 + r, 1:17, 0],
                                   gov[:, 1], crv[:, 1])

        for it in range(2):
            nc.sync.dma_start(out_d[it], hf[:].rearrange(
                "p (i x) -> p i x", i=2, x=512)[:, it, :])

    nc.compile()
    return nc


BT_W = np.array([[1, 0, -1, 0], [0, 1, 1, 0], [0, -1, 1, 0], [0, 1, 0, -1]],
                np.float32)
G_W = np.array([[1, 0, 0], [.5, .5, .5], [.5, -.5, .5], [0, 0, 1]], np.float32)

GATE_PERM = [0, 2, 3, 1]  # reorder [i, f, o, g] -> [i, o, g, f]


def _f8(a, scale):
    return np.clip(np.asarray(a, dtype=np.float32) * scale,
                   -240.0, 240.0).astype(ml_dtypes.float8_e4m3)


def _prep_weights(wx, wh, flip):
    ws = np.stack([np.asarray(wx), np.asarray(wh)])  # [2, 1024, 256, 3, 3]
    if flip:
        ws = ws[:, :, :, ::-1, :]
    # [cv, gate, ht, ch, it, ic, dy, dx]
    ws = ws.reshape(2, 4, 2, 128, 2, 128, 3, 3)[:, GATE_PERM]
    # U = G w over dx
    # u8: [cv, dy, pos, ic, it, (gate, ht, ch)]
    u8 = np.einsum('pa,cgemtida->cdpitgem', G_W, ws).reshape(2, 12, 128, 2, 1024)
    u8 = np.concatenate([_f8(u8[0], WXS), _f8(u8[1], WHS)])  # [24,128,2,1024]
    # u16: [cv, dy, pos, it, ic, (gate, ht, ch)]
    u16 = np.einsum('pa,cgemtida->cdptigem', G_W, ws).reshape(48, 128, 1024)
    return u8, u16.astype(ml_dtypes.bfloat16)


def _prep_x(xb, flip):
    # xb: [T, 256, 32, 32] for one batch element
    xc = np.asarray(xb, dtype=np.float32)
    if flip:
        xc = xc[:, :, ::-1, :]
    buf = np.zeros((T, 2, 128, ROWS, WC), dtype=np.float32)
    for it in range(2):
        buf[:, it, :, 1:25, 1:33] = xc[:, it * 128:(it + 1) * 128, 0:24, :]
    # V_x = B^T x over 4-col windows at stride 2
    cols = np.arange(16)[:, None] * 2 + np.arange(4)[None, :]
    xw = buf[:, :, :, :VR8, cols]            # [T, 2, 128, VR8, 16, 4]
    vx = np.einsum('pa,tiwvja->twipvj', BT_W, xw)  # [T, 128, 2, 4, VR8, 16]
    vx8 = _f8(vx[:NF8], XS).reshape(NF8, 128, 2 * 4 * VR8 * 16)
    vx16 = np.ascontiguousarray(vx[NF8:, :, :, :, :VR16, :]).astype(
        ml_dtypes.bfloat16).reshape(T - NF8, 128, 2 * 4 * VR16 * 16)
    return vx8, vx16


def kernel(x, wx, wh, bh):
    x = np.asarray(x, dtype=np.float32)
    B = x.shape[0]
    bias = np.ascontiguousarray(
        np.asarray(bh, dtype=np.float32).reshape(4, 2, 128)[GATE_PERM]
        .transpose(2, 0, 1).reshape(128, 8))

    u8_lo, u16_lo = _prep_weights(wx, wh, flip=False)
    u8_hi, u16_hi = _prep_weights(wx, wh, flip=True)

    in_maps = []
    for c in range(N_CORES):
        b, half = c // 2, c % 2
        vx8, vx16 = _prep_x(x[b], flip=bool(half))
        in_maps.append({
            "vx8": vx8,
            "vx16": vx16,
            "u8": u8_hi if half else u8_lo,
            "u16": u16_hi if half else u16_lo,
            "bias": bias,
        })

    if "nc" not in _cache:
        _cache["nc"] = _build_nc()
    nc = _cache["nc"]

    res = run_bass_kernel_spmd(nc, in_maps, core_ids=list(range(N_CORES)))
    _cache["last_results"] = res

    out = np.zeros((B, 256, 32, 32), dtype=np.float32)
    for c in range(N_CORES):
        b, half = c // 2, c % 2
        hq = res.results[c]["hout"].reshape(2, 128, 2, 16, 16)
        h = np.zeros((2, 128, 16, 32), dtype=np.float32)
        h[:, :, :, 0::2] = hq[:, :, 0]
        h[:, :, :, 1::2] = hq[:, :, 1]
        h = np.concatenate([h[0], h[1]], axis=0)  # [256, 16, 32]
        if half:
            out[b, :, 16:32, :] = h[:, ::-1, :]
        else:
            out[b, :, 0:16, :] = h
    return out


# revision 23
# speedup vs baseline: 1.0958x; 1.0958x over previous
"""ConvLSTM (B=4, T=8, C=HID=256, H=W=32, 3x3 SAME convs) on 8 TRN2 NeuronCores.

Sharding: data-parallel over batch (4) x spatial halves of H (2) = 8 cores,
zero inter-core communication. Each core computes its half's rows plus a
shrinking halo margin: at step t it computes 23-t rows; wrong values erode
inward from the un-owned edge at 1 row/step, leaving exactly the owned 16
rows correct after T=8 steps. Upper halves are row-flipped host-side (with
dy-flipped kernels) so all 8 cores run the same SPMD instruction stream.

Compute: 1D Winograd F(2,3) along W for both convs at every step: per
output-channel octile, 3(dy) x 4(pos) matmuls over K=ic accumulate four
position planes M_p, and VectorE applies A^T ([m0+m1+m2, m1-m2-m3]) to
produce the 16 even/odd column pairs - 1.5x fewer PE columns than direct
conv, and every step fits a single 512-col PSUM chunk.

Precision: steps 0..3 quantize V (data transform) and U (weight transform)
to fp8(e4m3) and run DoubleRow matmuls (ic-pair, K=256, 2x PE rate); steps
4..7 run bf16. fp8 errors injected at early steps decay through the forget
gate; simulated end-to-end rel err ~1.5e-2 vs the 2e-2 budget. Scales keep
e4m3 in range (x*16/wx*512, h*8/wh*1024 - both products 8192, undone by the
activation scale). The h data transform runs on VectorE (fp8 steps) or
GpSimd (bf16 steps); the x transform is precomputed host-side.
"""
import numpy as np
import ml_dtypes
from contextlib import ExitStack

import concourse.bass as bass
import concourse.tile as tile
from concourse import bacc, mybir
from concourse.bass_utils import run_bass_kernel_spmd

F8 = mybir.dt.float8e4
BF16 = mybir.dt.bfloat16
F32 = mybir.dt.float32
AF = mybir.ActivationFunctionType
ALU = mybir.AluOpType
DR = mybir.MatmulPerfMode.DoubleRow

N_CORES = 8
T = 8
NF8 = 4            # steps 0..NF8-1 run fp8 Winograd; the rest bf16 Winograd
ROWS = 26          # h plane rows: p=0 is the y=-1 zero row, p=1..24 = y=0..23
WC = 34            # padded width
PLANE = ROWS * WC  # 884
CROWS = 23
CPL = CROWS * 32
VR8 = 25           # V rows, fp8 steps (t=0 reads dy..dy+22, dy<=2)
VR16 = 21          # V rows, bf16 steps (t=4 reads dy..dy+18)

XS, WXS = 16.0, 512.0
HS, WHS = 8.0, 1024.0
DESCALE = 1.0 / 8192.0

_cache = {}


def _build_nc():
    nc = bacc.Bacc("TRN2", target_bir_lowering=False, debug=False,
                   num_devices=N_CORES)
    vx8_d = nc.dram_tensor("vx8", [NF8, 128, 2 * 4 * VR8 * 16], F8,
                           kind="ExternalInput").ap()
    vx16_d = nc.dram_tensor("vx16", [T - NF8, 128, 2 * 4 * VR16 * 16], BF16,
                            kind="ExternalInput").ap()
    u8_d = nc.dram_tensor("u8", [24, 128, 2, 1024], F8, kind="ExternalInput").ap()
    u16_d = nc.dram_tensor("u16", [48, 128, 1024], BF16, kind="ExternalInput").ap()
    b_d = nc.dram_tensor("bias", [128, 8], F32, kind="ExternalInput").ap()
    out_d = nc.dram_tensor("hout", [2, 128, 512], F32, kind="ExternalOutput").ap()

    with tile.TileContext(nc) as tc, ExitStack() as ctx:
        wp = ctx.enter_context(tc.tile_pool(name="wp", bufs=1))
        vxp8 = ctx.enter_context(tc.tile_pool(name="vxp8", bufs=2))
        vxp16 = ctx.enter_context(tc.tile_pool(name="vxp16", bufs=2))
        vhp8 = ctx.enter_context(tc.tile_pool(name="vhp8", bufs=1))
        vhp16 = ctx.enter_context(tc.tile_pool(name="vhp16", bufs=1))
        hp = ctx.enter_context(tc.tile_pool(name="hp", bufs=1))
        cp = ctx.enter_context(tc.tile_pool(name="cp", bufs=1))
        bp = ctx.enter_context(tc.tile_pool(name="bp", bufs=1))
        gp = ctx.enter_context(tc.tile_pool(name="gp", bufs=8))
        zp = ctx.enter_context(tc.tile_pool(name="zp", bufs=2))
        wtp = ctx.enter_context(tc.tile_pool(name="wtp", bufs=5))
        tp = ctx.enter_context(tc.tile_pool(name="tp", bufs=2))
        pp = ctx.enter_context(tc.tile_pool(name="pp", bufs=8, space="PSUM"))

        bt = bp.tile([128, 8], F32, tag="bias")
        nc.sync.dma_start(bt[:], b_d[:])

        h16a = hp.tile([128, 2 * PLANE], BF16, tag="h16a")
        h16b = hp.tile([128, 2 * PLANE], BF16, tag="h16b")
        hf = hp.tile([128, 1024], F32, tag="hf")
        ct = cp.tile([128, 2 * CPL], F32, tag="c")
        nc.vector.memset(ct[:], 0.0)
        nc.vector.memset(h16a[:], 0.0)
        nc.vector.memset(h16b[:], 0.0)

        vx0 = vxp8.tile([128, 2 * 4 * VR8 * 16], F8, tag="vx8")
        nc.gpsimd.dma_start(vx0[:], vx8_d[0])

        # fp8 weight tiles, one per (dy, pos). x-conv i/o/g columns first
        # (t=0 skips f octiles and h-convs), issued in consumption order.
        u8x = [wp.tile([128, 2, 768], F8, tag=f"u8x{j}", name=f"u8x{j}")
               for j in range(12)]
        u8xf = [wp.tile([128, 2, 256], F8, tag=f"u8xf{j}", name=f"u8xf{j}")
                for j in range(12)]
        u8h = [wp.tile([128, 2, 1024], F8, tag=f"u8h{j}", name=f"u8h{j}")
               for j in range(12)]
        for j in range(12):
            nc.sync.dma_start(u8x[j][:], u8_d[j][:, :, :768])
        for j in range(12):
            nc.sync.dma_start(u8h[j][:], u8_d[12 + j])
        for j in range(12):
            nc.sync.dma_start(u8xf[j][:], u8_d[j][:, :, 768:])
        u16 = [wp.tile([128, 1024], BF16, tag=f"u16_{j}", name=f"u16_{j}")
               for j in range(48)]
        for j in range(48):
            nc.sync.dma_start(u16[j][:], u16_d[j])

        def u8slice(cv, dy, pos, o):
            j = dy * 4 + pos
            if cv == 0:
                if o < 6:
                    return u8x[j][:, :, o * 128:(o + 1) * 128]
                return u8xf[j][:, :, (o - 6) * 128:(o - 5) * 128]
            return u8h[j][:, :, o * 128:(o + 1) * 128]

        def u16slice(cv, dy, pos, it, o):
            j = ((cv * 3 + dy) * 4 + pos) * 2 + it
            return u16[j][:, o * 128:(o + 1) * 128]

        hbufs = [h16a, h16b]

        for t in range(T):
            fp8 = t < NF8
            r = 23 - t
            n, n2 = r * 32, r * 16
            VR = VR8 if fp8 else VR16
            if t == 0:
                vx = vx0
            elif fp8:
                vx = vxp8.tile([128, 2 * 4 * VR8 * 16], F8, tag="vx8")
                nc.gpsimd.dma_start(vx[:], vx8_d[t])
            else:
                vx = vxp16.tile([128, 2 * 4 * VR16 * 16], BF16, tag="vx16")
                nc.gpsimd.dma_start(vx[:], vx16_d[t - NF8])
            vxv = vx[:].rearrange("p (i s v j) -> p i s v j", i=2, s=4, v=VR, j=16)

            h_in = hbufs[t % 2]
            h_out = hbufs[(t + 1) % 2] if t < T - 1 else None

            # data transform for the h-conv: V = B^T h per 4-col window
            if t > 0:
                if fp8:
                    vh = vhp8.tile([128, 2 * 4 * VR8 * 16], F8, tag="vh8")
                else:
                    vh = vhp16.tile([128, 2 * 4 * VR16 * 16], BF16, tag="vh16")
                vhv = vh[:].rearrange("p (i s v j) -> p i s v j", i=2, s=4, v=VR, j=16)
                hw = h_in[:].rearrange("p (i v c two) -> p i v c two",
                                       i=2, v=ROWS, c=17, two=2)
                d0 = hw[:, :, 0:VR, 0:16, 0]
                d1 = hw[:, :, 0:VR, 0:16, 1]
                d2 = hw[:, :, 0:VR, 1:17, 0]
                d3 = hw[:, :, 0:VR, 1:17, 1]
                eng = nc.vector if fp8 else nc.gpsimd
                eng.tensor_sub(vhv[:, :, 0], d0, d2)
                eng.tensor_add(vhv[:, :, 1], d1, d2)
                eng.tensor_sub(vhv[:, :, 2], d2, d1)
                eng.tensor_sub(vhv[:, :, 3], d1, d3)

            # final h tile stays parity-deinterleaved; the host re-interleaves
            hov = (None if h_out is not None else
                   hf[:].rearrange("p (i e v j) -> p i e v j",
                                   i=2, e=2, v=16, j=16))

            octs = [0, 1, 2, 3, 4, 5] if t == 0 else list(range(8))

            def x_mms(o, ps4):
                for dy in range(3):
                    for pos in range(4):
                        if fp8:
                            nc.tensor.matmul(
                                ps4[pos][:], u8slice(0, dy, pos, o),
                                vxv[:, :, pos, dy:dy + r, :],
                                start=(dy == 0), stop=(t == 0 and dy == 2),
                                perf_mode=DR, skip_group_check=True)
                        else:
                            for it in range(2):
                                nc.tensor.matmul(
                                    ps4[pos][:], u16slice(0, dy, pos, it, o),
                                    vxv[:, it, pos, dy:dy + r, :],
                                    start=(dy == 0 and it == 0), stop=False,
                                    skip_group_check=True)

            def h_mms(o, ps4):
                for dy in range(3):
                    for pos in range(4):
                        if fp8:
                            nc.tensor.matmul(
                                ps4[pos][:], u8slice(1, dy, pos, o),
                                vhv[:, :, pos, dy:dy + r, :],
                                start=False, stop=(dy == 2),
                                perf_mode=DR, skip_group_check=True)
                        else:
                            for it in range(2):
                                nc.tensor.matmul(
                                    ps4[pos][:], u16slice(1, dy, pos, it, o),
                                    vhv[:, it, pos, dy:dy + r, :],
                                    start=False, stop=(dy == 2 and it == 1),
                                    skip_group_check=True)

            gts = {}

            def drain(o, ps4):
                # z = A^T M: z_even = m0+m1+m2, z_odd = m1-m2-m3. Gates, z
                # and c all live in parity-deinterleaved layout (even block
                # then odd block) so every op here is flat/contiguous; only
                # the h-plane write re-interleaves. One PSUM operand per DVE
                # op: m1/m2 staged through ScalarE; t23 runs on GpSimd.
                zt = zp.tile([128, n], BF16, tag="z")
                s1 = wtp.tile([128, n2], BF16, tag="t01")
                s2 = wtp.tile([128, n2], BF16, tag="t01")
                t01 = wtp.tile([128, n2], BF16, tag="t01")
                t23 = wtp.tile([128, n2], BF16, tag="t01")
                nc.scalar.activation(s1[:], ps4[1][:], AF.Copy)
                nc.scalar.activation(s2[:], ps4[2][:], AF.Copy)
                nc.vector.tensor_add(t01[:], s1[:], ps4[0][:])
                nc.vector.tensor_add(zt[:, :n2], t01[:], ps4[2][:])
                nc.gpsimd.tensor_sub(t23[:], s1[:], s2[:])
                nc.vector.tensor_sub(zt[:, n2:], t23[:], ps4[3][:])
                gt = gp.tile([128, n], BF16, tag="g")
                gts[o] = gt
                func = AF.Relu if o in (4, 5) else AF.Sigmoid
                nc.scalar.activation(gt[:], zt[:], func, bias=bt[:, o:o + 1],
                                     scale=DESCALE if fp8 else 1.0)

            def alloc4():
                return [pp.tile([128, n2], F32, tag="ps", name=f"ps{i}")
                        for i in range(4)]

            ps_map = {}
            ps_map[octs[0]] = alloc4()
            ps_map[octs[1]] = alloc4()
            x_mms(octs[0], ps_map[octs[0]])
            x_mms(octs[1], ps_map[octs[1]])
            for idx, o in enumerate(octs):
                if t > 0:
                    h_mms(o, ps_map[o])
                drain(o, ps_map[o])
                del ps_map[o]
                if idx + 2 < len(octs):
                    nxt = octs[idx + 2]
                    ps_map[nxt] = alloc4()
                    x_mms(nxt, ps_map[nxt])

            # state update; halves run on different engines in parallel.
            # c holds the parity-deinterleaved layout at fixed stride;
            # gate/cr tiles are step-sized so views reconcile the strides.
            ctv = ct[:].rearrange("p (h e v j) -> p h e v j",
                                  h=2, e=2, v=CROWS, j=16)
            for hi in range(2):
                eng = nc.vector if hi == 0 else nc.gpsimd
                gi, go, gg = gts[0 + hi], gts[2 + hi], gts[4 + hi]

                def dv(x):
                    return x[:].rearrange("p (e v j) -> p e v j",
                                          e=2, v=r, j=16)
                cs = ctv[:, hi, :, 0:r, :]
                if t == 0:
                    eng.tensor_mul(cs, dv(gi), dv(gg))
                else:
                    gf = gts[6 + hi]
                    eng.tensor_mul(gg[:], gi[:], gg[:])
                    eng.tensor_mul(cs, dv(gf), cs)
                    eng.tensor_add(cs, cs, dv(gg))
                cr = tp.tile([128, n], BF16, tag="cr")
                if t < NF8 - 1:
                    # next step's conv consumes h in fp8 scaled by HS
                    eng.tensor_scalar(dv(cr), cs, 0.0, HS, ALU.max, ALU.mult)
                else:
                    eng.tensor_scalar_max(dv(cr), cs, 0.0)
                crv, gov = dv(cr), dv(go)
                if t == T - 1:
                    for e in range(2):
                        eng.tensor_mul(hov[:, hi, e, :, :],
                                       gov[:, e], crv[:, e])
                else:
                    hw2 = h_out[:].rearrange("p (i v ch two) -> p i v ch two",
                                             i=2, v=ROWS, ch=17, two=2)
                    eng.tensor_mul(hw2[:, hi, 1:1 + r, 0:16, 1],
                                   gov[:, 0], crv[:, 0])
                    eng.tensor_mul(hw2[:, hi, 1:1 + r, 1:17, 0],
                                   gov[:, 1], crv[:, 1])

        for it in range(2):
            nc.sync.dma_start(out_d[it], hf[:].rearrange(
                "p (i x) -> p i x", i=2, x=512)[:, it, :])

    nc.compile()
    return nc


BT_W = np.array([[1, 0, -1, 0], [0, 1, 1, 0], [0, -1, 1, 0], [0, 1, 0, -1]],
                np.float32)
G_W = np.array([[1, 0, 0], [.5, .5, .5], [.5, -.5, .5], [0, 0, 1]], np.float32)

GATE_PERM = [0, 2, 3, 1]  # reorder [i, f, o, g] -> [i, o, g, f]


def _f8(a, scale):
    return np.clip(np.asarray(a, dtype=np.float32) * scale,
                   -240.0, 240.0).astype(ml_dtypes.float8_e4m3)


def _prep_weights(wx, wh, flip):
    ws = np.stack([np.asarray(wx), np.asarray(wh)])  # [2, 1024, 256, 3, 3]
    if flip:
        ws = ws[:, :, :, ::-1, :]
    # [cv, gate, ht, ch, it, ic, dy, dx]
    ws = ws.reshape(2, 4, 2, 128, 2, 128, 3, 3)[:, GATE_PERM]
    # U = G w over dx
    # u8: [cv, dy, pos, ic, it, (gate, ht, ch)]
    u8 = np.einsum('pa,cgemtida->cdpitgem', G_W, ws).reshape(2, 12, 128, 2, 1024)
    u8 = np.concatenate([_f8(u8[0], WXS), _f8(u8[1], WHS)])  # [24,128,2,1024]
    # u16: [cv, dy, pos, it, ic, (gate, ht, ch)]
    u16 = np.einsum('pa,cgemtida->cdptigem', G_W, ws).reshape(48, 128, 1024)
    return u8, u16.astype(ml_dtypes.bfloat16)


def _prep_x(xb, flip):
    # xb: [T, 256, 32, 32] for one batch element
    xc = np.asarray(xb, dtype=np.float32)
    if flip:
        xc = xc[:, :, ::-1, :]
    buf = np.zeros((T, 2, 128, ROWS, WC), dtype=np.float32)
    for it in range(2):
        buf[:, it, :, 1:25, 1:33] = xc[:, it * 128:(it + 1) * 128, 0:24, :]
    # V_x = B^T x over 4-col windows at stride 2
    cols = np.arange(16)[:, None] * 2 + np.arange(4)[None, :]
    xw = buf[:, :, :, :VR8, cols]            # [T, 2, 128, VR8, 16, 4]
    vx = np.einsum('pa,tiwvja->twipvj', BT_W, xw)  # [T, 128, 2, 4, VR8, 16]
    vx8 = _f8(vx[:NF8], XS).reshape(NF8, 128, 2 * 4 * VR8 * 16)
    vx16 = np.ascontiguousarray(vx[NF8:, :, :, :, :VR16, :]).astype(
        ml_dtypes.bfloat16).reshape(T - NF8, 128, 2 * 4 * VR16 * 16)
    return vx8, vx16


def kernel(x, wx, wh, bh):
    x = np.asarray(x, dtype=np.float32)
    B = x.shape[0]
    bias = np.ascontiguousarray(
        np.asarray(bh, dtype=np.float32).reshape(4, 2, 128)[GATE_PERM]
        .transpose(2, 0, 1).reshape(128, 8))

    u8_lo, u16_lo = _prep_weights(wx, wh, flip=False)
    u8_hi, u16_hi = _prep_weights(wx, wh, flip=True)

    in_maps = []
    for c in range(N_CORES):
        b, half = c // 2, c % 2
        vx8, vx16 = _prep_x(x[b], flip=bool(half))
        in_maps.append({
            "vx8": vx8,
            "vx16": vx16,
            "u8": u8_hi if half else u8_lo,
            "u16": u16_hi if half else u16_lo,
            "bias": bias,
        })

    if "nc" not in _cache:
        _cache["nc"] = _build_nc()
    nc = _cache["nc"]

    res = run_bass_kernel_spmd(nc, in_maps, core_ids=list(range(N_CORES)))
    _cache["last_results"] = res

    out = np.zeros((B, 256, 32, 32), dtype=np.float32)
    for c in range(N_CORES):
        b, half = c // 2, c % 2
        hq = res.results[c]["hout"].reshape(2, 128, 2, 16, 16)
        h = np.zeros((2, 128, 16, 32), dtype=np.float32)
        h[:, :, :, 0::2] = hq[:, :, 0]
        h[:, :, :, 1::2] = hq[:, :, 1]
        h = np.concatenate([h[0], h[1]], axis=0)  # [256, 16, 32]
        if half:
            out[b, :, 16:32, :] = h[:, ::-1, :]
        else:
            out[b, :, 0:16, :] = h
    return out
